# revision 1
# baseline (speedup 1.0000x reference)
"""DAS (delay-and-sum) beamforming kernel for 8 Trainium2 NeuronCores.

Strategy
--------
image[b,c,p] = sum_s sensor_data[b,c,s, t[s,p]]  with t a per-(sensor,pixel)
delay index into the 2048-sample trace, t in [0, 1867).

Sharding: sensors are split 16 per NeuronCore (8 cores x 16 = 128); each core
computes a partial image [8ch, 512*512] (8 channels = 4 batch x 2 comp) summed
over its 16 sensors; the host sums the 8 partial images (the unshard step).

Per core, for each 1024-pixel tile:
 - GPSIMD IndirectCopy gathers, for 8 sensors in parallel (one per Q7 core),
   1024 pixels from each sensor's 8 channel traces (the 16 SBUF partitions of
   a Q7 core share one index stream; 8 carry that sensor's channel traces).
   Two calls cover the core's 16 sensors (two groups of 8).
 - A [128->8] fp32 matmul on the PE (weights select partition j==c of each
   16-row group) reduces the 128 gathered rows to the 8 channel sums,
   accumulating both sensor groups in PSUM rows 0-7.
 - ACT copies PSUM [8,512]x2 into an [8, 16384] staging tile (partition
   starts equal and 0 => legal), and every 16 tiles the staging block is
   DMA'd to the HBM output slice (double-buffered).

Delay indices are computed on the host in numpy float32 with the exact op
sequence of the reference's _delay_indices; this is bit-identical to the
reference evaluated with jax on CPU (verified), honoring the truncating
int cast the reference documents.
"""

import numpy as np

import concourse.bass as bass
import concourse.mybir as mybir
from concourse.bass_utils import run_bass_kernel_spmd

F32 = mybir.dt.float32
U16 = mybir.dt.uint16

NX, NY = 512, 512
DX, DY = 1e-4, 1e-4
VS = 1550.0
DT = 2.5e-8

S = 128            # sensors
T = 2048           # trace length
NPX = NX * NY      # 262144 pixels
NCORES = 8
SPC = 16           # sensors per NeuronCore
TILE = 1024        # pixels per indirect_copy call (ISA cap: 1024 dst elems)
NTILES = NPX // TILE          # 256
BLK = 8                       # tiles per staging block
NBLK = NTILES // BLK          # 32 staging blocks
STG_F = BLK * TILE            # 8192 staging elems per partition
IDX_F = NPX // 16             # 16384 idx elems per partition per group


def _delay_indices(sensor_xy: np.ndarray) -> np.ndarray:
    """Replicates the reference's jax ops in numpy float32 (bit-identical on
    CPU: sub, add, mul, square, sqrt, div are all correctly rounded)."""
    ix = np.arange(1, NX + 1, dtype=np.float32)
    iy = np.arange(1, NY + 1, dtype=np.float32)
    x = sensor_xy[:, 0].astype(np.float32)
    y = sensor_xy[:, 1].astype(np.float32)
    dx = (x[:, None] - ix[None, :] + np.float32(1.0)) * np.float32(DX)
    dy = (y[:, None] - iy[None, :] + np.float32(1.0)) * np.float32(DY)
    dis = np.sqrt(dx[:, :, None] ** 2 + dy[:, None, :] ** 2)
    t = (dis / np.float32(VS) / np.float32(DT)).astype(np.int32)
    return t.reshape(sensor_xy.shape[0], -1)


def build_nc(repeat: int = 1, nv: int = TILE) -> bass.Bass:
    nc = bass.Bass()
    w = nc.declare_dram_parameter("w", [128, 8], F32, isOutput=False)
    d0 = nc.declare_dram_parameter("d0", [128, T], F32, isOutput=False)
    d1 = nc.declare_dram_parameter("d1", [128, T], F32, isOutput=False)
    idx = nc.declare_dram_parameter("idx", [128, 2 * IDX_F], U16, isOutput=False)
    out = nc.declare_dram_parameter("out", [8, NPX], F32, isOutput=True)

    with (
        nc.sbuf_tensor("w_sb", [128, 8], F32) as w_sb,
        nc.sbuf_tensor("d0_sb", [128, T], F32) as d0_sb,
        nc.sbuf_tensor("d1_sb", [128, T], F32) as d1_sb,
        nc.sbuf_tensor("i_sb", [128, 2 * IDX_F], U16) as i_sb,
        nc.sbuf_tensor("oA0", [128, TILE], F32) as oA0,
        nc.sbuf_tensor("oA1", [128, TILE], F32) as oA1,
        nc.sbuf_tensor("oB0", [128, TILE], F32) as oB0,
        nc.sbuf_tensor("oB1", [128, TILE], F32) as oB1,
        nc.sbuf_tensor("stg0", [8, STG_F], F32) as stg0,
        nc.sbuf_tensor("stg1", [8, STG_F], F32) as stg1,
        nc.psum_tensor("ps00", [8, 512], F32) as ps00,
        nc.psum_tensor("ps01", [8, 512], F32) as ps01,
        nc.psum_tensor("ps10", [8, 512], F32) as ps10,
        nc.psum_tensor("ps11", [8, 512], F32) as ps11,
        nc.semaphore("dsem") as dsem,
        nc.semaphore("g_done") as g_done,
        nc.semaphore("m_done") as m_done,
        nc.semaphore("c_done") as c_done,
        nc.semaphore("o_sem") as o_sem,
        nc.Block() as block,
    ):
        oA = [oA0, oA1]
        oB = [oB0, oB1]
        ps = [[ps00, ps01], [ps10, ps11]]
        stg = [stg0, stg1]
        total_t = repeat * NTILES
        total_b = repeat * NBLK

        @block.sync
        def _(sync):
            sync.dma_start(w_sb[:, :], w[:, :]).then_inc(dsem, 16)
            sync.dma_start(d0_sb[:, :], d0[:, :]).then_inc(dsem, 16)
            sync.dma_start(d1_sb[:, :], d1[:, :]).then_inc(dsem, 16)
            sync.dma_start(i_sb[:, :], idx[:, :]).then_inc(dsem, 16)
            for b in range(total_b):
                pb = b % 2
                sync.wait_ge(c_done, BLK * (b + 1))
                sync.dma_start(
                    bass.AP(out, STG_F * (b % NBLK), [[NPX, 8], [1, STG_F]]),
                    stg[pb][:, :],
                ).then_inc(o_sem, 16)
            sync.wait_ge(o_sem, 16 * total_b)

        @block.gpsimd
        def _(g):
            g.wait_ge(dsem, 64)
            for gi in range(total_t):
                i = gi % NTILES
                p = gi % 2
                if gi >= 2:
                    g.wait_ge(m_done, gi - 1)
                g.indirect_copy(
                    oA[p][:, :nv], d0_sb[:, :],
                    i_sb[:, 64 * i:64 * i + nv // 16], True)
                g.indirect_copy(
                    oB[p][:, :nv], d1_sb[:, :],
                    i_sb[:, IDX_F + 64 * i:IDX_F + 64 * i + nv // 16], True,
                ).then_inc(g_done, 1)

        @block.tensor
        def _(tensor):
            tensor.wait_ge(dsem, 64)
            for gi in range(total_t):
                p = gi % 2
                tensor.wait_ge(g_done, gi + 1)
                if gi >= 2:
                    tensor.wait_ge(c_done, gi - 1)
                h = nv // 2
                tensor.matmul(ps[p][0][:, :h], w_sb[:, :], oA[p][:, 0:h],
                              start=True, stop=False)
                tensor.matmul(ps[p][0][:, :h], w_sb[:, :], oB[p][:, 0:h],
                              start=False, stop=True)
                tensor.matmul(ps[p][1][:, :h], w_sb[:, :], oA[p][:, h:2 * h],
                              start=True, stop=False)
                tensor.matmul(ps[p][1][:, :h], w_sb[:, :], oB[p][:, h:2 * h],
                              start=False, stop=True).then_inc(m_done, 1)

        @block.scalar
        def _(scalar):
            for gi in range(total_t):
                p = gi % 2
                b = gi // BLK
                pb = b % 2
                scalar.wait_ge(m_done, gi + 1)
                if b >= 2 and gi % BLK == 0:
                    scalar.wait_ge(o_sem, 16 * (b - 1))
                f0 = (gi % BLK) * TILE
                h = nv // 2
                scalar.copy(stg[pb][:, f0:f0 + h], ps[p][0][:, :h])
                scalar.copy(stg[pb][:, f0 + 512:f0 + 512 + h],
                            ps[p][1][:, :h]).then_inc(c_done, 1)

    return nc


_NC_CACHE: dict = {}


def _get_nc(repeat: int = 1, nv: int = TILE) -> bass.Bass:
    if (repeat, nv) not in _NC_CACHE:
        _NC_CACHE[(repeat, nv)] = build_nc(repeat, nv)
    return _NC_CACHE[(repeat, nv)]


def make_in_maps(sensor_data: np.ndarray, t_u16: np.ndarray):
    """Per-core input dicts. t_u16: [128 sensors, NPX] uint16."""
    sd = np.asarray(sensor_data, dtype=np.float32)        # (4, 2, 128, 2048)
    traces = sd.transpose(2, 0, 1, 3).reshape(S, 8, T)    # (s, c=(b,c2), T)
    # W[16k+j, c] = (j == c), j < 8: selects channel c of each sensor group,
    # zeroing the replica rows (j >= 8).
    w = np.zeros((128, 8), np.float32)
    for k in range(8):
        for j in range(8):
            w[16 * k + j, j] = 1.0
    in_maps = []
    for n in range(NCORES):
        tn = t_u16[SPC * n:SPC * (n + 1)]                 # (16, NPX)
        idx = np.empty((128, 2 * IDX_F), np.uint16)
        d = np.empty((2, 128, T), np.float32)
        for g in range(2):
            for k in range(8):
                s_loc = 8 * g + k
                # tile i covers px [1024*i, 1024*(i+1)); stream wrapped over
                # the core's 16 partitions (partition index fastest).
                blk = (tn[s_loc].reshape(NTILES, 64, 16)
                       .transpose(2, 0, 1).reshape(16, IDX_F))
                idx[16 * k:16 * k + 16, g * IDX_F:(g + 1) * IDX_F] = blk
                for j in range(16):
                    d[g, 16 * k + j] = traces[SPC * n + s_loc, j % 8]
        in_maps.append({"w": w, "d0": d[0], "d1": d[1], "idx": idx})
    return in_maps


def kernel(sensor_data: np.ndarray, sensor_xy: np.ndarray) -> np.ndarray:
    t = _delay_indices(np.asarray(sensor_xy))
    t_u16 = t.astype(np.uint16)
    in_maps = make_in_maps(sensor_data, t_u16)
    nc = _get_nc(1)
    res = run_bass_kernel_spmd(nc, in_maps, list(range(NCORES)))
    acc = np.zeros((8, NPX), np.float64)
    for r in res.results:
        acc += r["out"]
    return acc.astype(np.float32).reshape(4, 2, NX, NY)

